# revision 15
# baseline (speedup 1.0000x reference)
"""Trainium2 Bass kernel for nn_CE_15238543966409 (sparse patch attention).

Contract: kernel(**inputs) takes the FULL unsharded inputs (see shapes below)
and returns the FULL [2, 64, 256, 256] float32 output.

Sharding: data-parallel over batch x query-patches. Core c handles image
c//4 and query patches [1024*(c%4), 1024*(c%4+1)) of the L=4096 patch grid.
The small convs / unfold / fold / final 1x1 conv run on host in numpy
(~3% of FLOPs); the two large attention matmuls, the fc layers and the
masked softmax (~97% of FLOPs) run on the 8 NeuronCores.

Device algorithm (per core), algebraically identical to the reference:
  - scoreT[m, q] = xi_f[m] . wi_f[q] - thr[q] (threshold fused in as an
    extra contraction row; [m,q] layout so no transpose is needed anywhere)
  - e = exp(10*t - C);  hm = (e >= exp(-C)) * e   (one scalar_tensor_tensor)
    The per-query bias term and the softmax max-subtraction cancel exactly
    in the renormalized attention, so neither is computed. C is a global
    shift keeping exp in fp32 range (10*t reaches ~120).
  - aggT[d, q] = sum_m pi[m, d] * hm[m, q]; a ones-column appended to pi
    yields S_m = sum_m hm in the same matmul. attn normalization is pulled
    out past the matmul: aggT *= 1/max(S_m, 1e-30).
  - The dropped 1e-8*S_all term is bounded: masked-in e >= exp(-C) so the
    relative denominator error is <= 1e-8 * (1 + 4096) ~= 4e-5.
"""

import numpy as np
import ml_dtypes

BF16 = ml_dtypes.bfloat16

KS, ST, SC = 7, 4, 10.0
B, CIN, H, W = 2, 64, 256, 256
CI = 16
LH = LW = 64
L = LH * LW                      # 4096
D_IN = KS * KS * CI              # 784
D_OUT = D_IN // 4                # 196
PT, PB, PL, PR = 1, 2, 1, 2      # 'same' pad for 256/7/4
C_SHIFT = 45.0                   # global exp shift (attn-invariant)
N_CORES = 8
QS = L // 4                      # 1024 queries per core
KT = 7                           # k-tiles of 112 over D_IN
KTS = D_IN // KT                 # 112
DT = 7                           # d-tiles of 113 over D_IN+1 (ones col)
DTS = 113
D_PAD = DT * DTS                 # 791
N_MT = 32                        # m tiles of 128
N_QC = 2                         # q chunks of 512
QCS = 512

_compiled = {}
last_results = None  # BassKernelResults of the most recent run (for profiling)


# ---------------------------------------------------------------- host math
def _conv3x3(x, w, bias):
    xp = np.pad(x, ((0, 0), (0, 0), (1, 1), (1, 1)))
    n, _, h, ww = x.shape
    y = np.zeros((n, w.shape[0], h, ww), np.float32)
    for dh in range(3):
        for dw in range(3):
            y += np.einsum('oc,nchw->nohw', w[:, :, dh, dw],
                           xp[:, :, dh:dh + h, dw:dw + ww], optimize=True)
    return y + bias[None, :, None, None]


def _conv1x1(x, w, bias):
    return np.einsum('oc,nchw->nohw', w[:, :, 0, 0], x,
                     optimize=True) + bias[None, :, None, None]


def _patch_view(x):
    """[B,C,H,W] -> strided view [B, C, LH, LW, KS, KS] on the padded grid."""
    xp = np.pad(x, ((0, 0), (0, 0), (PT, PB), (PL, PR)))
    s = xp.strides
    return np.lib.stride_tricks.as_strided(
        xp, (x.shape[0], x.shape[1], LH, LW, KS, KS),
        (s[0], s[1], s[2] * ST, s[3] * ST, s[2], s[3]))


def _unfold_T(x):
    """-> [B, C*KS*KS, L] feature-major (c, kh, kw) rows, (i, j) cols."""
    v = _patch_view(x)
    return np.ascontiguousarray(v.transpose(0, 1, 4, 5, 2, 3)).reshape(
        x.shape[0], -1, L)


def _unfold(x):
    """-> [B, L, C*KS*KS] patch-major."""
    v = _patch_view(x)
    return np.ascontiguousarray(v.transpose(0, 2, 3, 1, 4, 5)).reshape(
        x.shape[0], L, -1)


def _fold(agg):
    """agg [B, L, D_IN] -> zi [B, CI, H, W] (overlap-add / count, crop)."""
    p6 = agg.reshape(B, LH, LW, CI, KS, KS)
    hp, wp = H + PT + PB, W + PL + PR
    folded = np.zeros((B, CI, hp, wp), np.float32)
    cnt = np.zeros((hp, wp), np.float32)
    for kh in range(KS):
        for kw in range(KS):
            folded[:, :, kh:kh + ST * LH:ST, kw:kw + ST * LW:ST] += \
                p6[:, :, :, :, kh, kw].transpose(0, 3, 1, 2)
            cnt[kh:kh + ST * LH:ST, kw:kw + ST * LW:ST] += 1.0
    # (pl, pt) crop order matches the reference
    return (folded / cnt)[:, :, PL:PL + H, PT:PT + W]


# ---------------------------------------------------------------- bass build
def _build_bass():
    import concourse.bacc as bacc
    import concourse.mybir as mybir
    from concourse.tile import TileContext

    f32 = mybir.dt.float32
    f32r = mybir.dt.float32r
    bf16 = mybir.dt.bfloat16
    AF = mybir.ActivationFunctionType
    OP = mybir.AluOpType
    T_THR = float(np.exp(np.float32(-C_SHIFT)))

    nc = bacc.Bacc(None, target_bir_lowering=False, debug=False)
    wiT_d = nc.dram_tensor("wiT", [D_IN, L], bf16, kind="ExternalInput")
    pi_d = nc.dram_tensor("pi", [L, D_PAD], bf16, kind="ExternalInput")
    thr_d = nc.dram_tensor("thr", [1, QS], bf16, kind="ExternalInput")
    neg_d = nc.dram_tensor("negones", [1, L], bf16, kind="ExternalInput")
    f1w_d = nc.dram_tensor("fc1wT", [D_IN, D_OUT], bf16, kind="ExternalInput")
    f2w_d = nc.dram_tensor("fc2wT", [D_IN, D_OUT], bf16, kind="ExternalInput")
    f1b_d = nc.dram_tensor("fc1b", [D_OUT, 1], f32, kind="ExternalInput")
    f2b_d = nc.dram_tensor("fc2b", [D_OUT, 1], f32, kind="ExternalInput")
    out_d = nc.dram_tensor("aggT", [D_PAD, QS], f32, kind="ExternalOutput")

    with TileContext(nc) as tc:
        with tc.tile_pool(name="const", bufs=1) as const, \
             tc.tile_pool(name="wik", bufs=2) as wikp, \
             tc.tile_pool(name="xa", bufs=1) as xap, \
             tc.tile_pool(name="pip", bufs=1) as pip, \
             tc.tile_pool(name="hmp", bufs=1) as hmp, \
             tc.tile_pool(name="ep", bufs=3) as ep, \
             tc.tile_pool(name="outp", bufs=4) as outp, \
             tc.tile_pool(name="ps_mix", bufs=4, space="PSUM") as ps_mix, \
             tc.tile_pool(name="ps_agg", bufs=4, space="PSUM") as ps_agg:

            # ---- constants ----
            f1w_t, f2w_t = [], []
            for k in range(KT):
                t1 = const.tile([KTS, D_OUT], bf16, tag=f"f1w{k}", name=f"f1w{k}")
                nc.gpsimd.dma_start(out=t1, in_=f1w_d[k * KTS:(k + 1) * KTS, :])
                f1w_t.append(t1)
                t2 = const.tile([KTS, D_OUT], bf16, tag=f"f2w{k}", name=f"f2w{k}")
                f2w_t.append(t2)
            f1b0 = const.tile([128, 1], f32, tag="f1b0", name="f1b0")
            f1b1 = const.tile([68, 1], f32, tag="f1b1", name="f1b1")
            f2b0 = const.tile([128, 1], f32, tag="f2b0", name="f2b0")
            f2b1 = const.tile([68, 1], f32, tag="f2b1", name="f2b1")
            nc.gpsimd.dma_start(out=f1b0, in_=f1b_d[0:128, :])
            nc.gpsimd.dma_start(out=f1b1, in_=f1b_d[128:D_OUT, :])
            nc.gpsimd.dma_start(out=f2b0, in_=f2b_d[0:128, :])
            nc.gpsimd.dma_start(out=f2b1, in_=f2b_d[128:D_OUT, :])
            cshift = const.tile([128, 1], f32, tag="cshift", name="cshift")
            nc.vector.memset(cshift, -C_SHIFT)

            # ---- persistent activations ----
            xa0 = xap.tile([128, L], bf16, tag="xa0", name="xa0")
            xa1 = xap.tile([69, L], bf16, tag="xa1", name="xa1")
            wa0 = xap.tile([128, QS], bf16, tag="wa0", name="wa0")
            wa1 = xap.tile([69, QS], bf16, tag="wa1", name="wa1")
            nc.gpsimd.dma_start(out=xa1[68:69, :], in_=neg_d[:, :])
            nc.gpsimd.dma_start(out=wa1[68:69, :], in_=thr_d[:, :])

            # ---- fc2 (keys, all m) + fc1 (queries, cols 0:1024) ----
            mt_rows = [(0, 128), (128, D_OUT)]
            for mc in range(8):
                wik = wikp.tile([KTS, KT, QCS], bf16, tag="wik", name="wik")
                for k in range(KT):
                    if mc == 0:
                        nc.sync.dma_start(
                            out=f2w_t[k],
                            in_=f2w_d[k * KTS:(k + 1) * KTS, :])
                    nc.sync.dma_start(
                        out=wik[:, k, :],
                        in_=wiT_d[k * KTS:(k + 1) * KTS,
                                  mc * QCS:(mc + 1) * QCS])
                for mt, (r0, r1) in enumerate(mt_rows):
                    rows = r1 - r0
                    ps = ps_mix.tile([128, QCS], f32, tag="mix", name="psfc2")
                    for k in range(KT):
                        nc.tensor.matmul(
                            ps[:rows, :], f2w_t[k][:, r0:r1],
                            wik[:, k, :],
                            start=(k == 0), stop=(k == KT - 1))
                    xa = xa0 if mt == 0 else xa1
                    nc.scalar.activation(
                        xa[0:rows, mc * QCS:(mc + 1) * QCS], ps[:rows, :],
                        AF.Relu, bias=(f2b0 if mt == 0 else f2b1))
                if mc < 2:
                    for mt, (r0, r1) in enumerate(mt_rows):
                        rows = r1 - r0
                        ps = ps_mix.tile([128, QCS], f32, tag="mix", name="psfc1")
                        for k in range(KT):
                            nc.tensor.matmul(
                                ps[:rows, :], f1w_t[k][:, r0:r1],
                                wik[:, k, :],
                                start=(k == 0), stop=(k == KT - 1))
                        wa = wa0 if mt == 0 else wa1
                        nc.scalar.activation(
                            wa[0:rows, mc * QCS:(mc + 1) * QCS], ps[:rows, :],
                            AF.Relu, bias=(f1b0 if mt == 0 else f1b1))

            # ---- pi tiles (resident, bf16); emitted after the fc phase so
            # their DMAs don't delay the wik chunks the fc matmuls need ----
            pi_t = []
            for m in range(N_MT):
                t = pip.tile([128, D_PAD], bf16, tag=f"pi{m}", name=f"pi{m}")
                nc.gpsimd.dma_start(out=t, in_=pi_d[m * 128:(m + 1) * 128, :])
                pi_t.append(t)

            # ---- attention per q-chunk ----
            for qc in range(N_QC):
                qsl = slice(qc * QCS, (qc + 1) * QCS)
                hm_t = []
                for m in range(N_MT):
                    msl = slice(m * 128, (m + 1) * 128)
                    ps_t = ps_mix.tile([128, QCS], f32, tag="mix", name="pst")
                    nc.tensor.matmul(ps_t, xa0[:, msl], wa0[:, qsl],
                                     start=True, stop=False)
                    nc.tensor.matmul(ps_t, xa1[:, msl], wa1[:, qsl],
                                     start=False, stop=True)
                    e_t = ep.tile([128, QCS], f32, tag="e", name="e_t")
                    nc.scalar.activation(e_t, ps_t, AF.Exp,
                                         scale=SC, bias=cshift)
                    hm = hmp.tile([128, QCS], bf16, tag=f"hm{m}", name=f"hm{m}")
                    nc.vector.scalar_tensor_tensor(
                        hm, e_t, T_THR, e_t, op0=OP.is_ge, op1=OP.mult)
                    hm_t.append(hm)

                pa = {}
                for d in range(DT):
                    p = ps_agg.tile([DTS, QCS], f32, tag="agg", name="psagg")
                    pa[d] = p
                    dsl = slice(d * DTS, (d + 1) * DTS)
                    for m in range(N_MT):
                        nc.tensor.matmul(p, pi_t[m][:, dsl], hm_t[m],
                                         start=(m == 0), stop=(m == N_MT - 1))
                    ao = outp.tile([DTS, QCS], f32, tag="ao", name="ao")
                    nc.scalar.activation(ao, p, AF.Copy)
                    nc.sync.dma_start(out=out_d[dsl, qsl], in_=ao)
    nc.finalize()
    return nc


def _get_nc():
    if "nc" not in _compiled:
        _compiled["nc"] = _build_bass()
    return _compiled["nc"]


# ---------------------------------------------------------------- entry
def kernel(b, g_w, g_b, theta_w, theta_b, W_w, W_b, fc1_w, fc1_b,
           fc2_w, fc2_b, thr_w, thr_b, bias_w, bias_b):
    from concourse.bass_utils import run_bass_kernel_spmd

    b = np.asarray(b, np.float32)
    b1 = _conv3x3(b, np.asarray(g_w), np.asarray(g_b))
    b2 = _conv1x1(b, np.asarray(theta_w), np.asarray(theta_b))
    thr = (_unfold(b).reshape(B * L, -1) @ np.asarray(thr_w).reshape(-1)
           + np.asarray(thr_b)).reshape(B, L).astype(np.float32)

    wiT = _unfold_T(b1)                       # [B, 784, 4096] f32
    pim = _unfold(b2)                         # [B, 4096, 784] f32
    f1wT = np.ascontiguousarray(np.asarray(fc1_w, np.float32).T.astype(BF16))
    f2wT = np.ascontiguousarray(np.asarray(fc2_w, np.float32).T.astype(BF16))
    f1b = np.asarray(fc1_b, np.float32).reshape(D_OUT, 1)
    f2b = np.asarray(fc2_b, np.float32).reshape(D_OUT, 1)

    in_maps = []
    for c in range(N_CORES):
        img, qoff = c // 4, (c % 4) * QS
        wiT_c = np.concatenate(
            [wiT[img][:, qoff:], wiT[img][:, :qoff]], axis=1).astype(BF16)
        pi_c = np.zeros((L, D_PAD), BF16)
        pi_c[:, :D_IN] = np.concatenate(
            [pim[img][qoff:], pim[img][:qoff]], axis=0).astype(BF16)
        pi_c[:, D_IN] = BF16(1.0)
        in_maps.append({
            "wiT": np.ascontiguousarray(wiT_c),
            "pi": pi_c,
            "thr": np.ascontiguousarray(
                thr[img, qoff:qoff + QS][None, :].astype(BF16)),
            "negones": np.full((1, L), -1.0, BF16),
            "fc1wT": f1wT, "fc2wT": f2wT, "fc1b": f1b, "fc2b": f2b,
        })

    nc = _get_nc()
    try:
        res = run_bass_kernel_spmd(nc, in_maps, core_ids=list(range(N_CORES)))
    except ModuleNotFoundError:
        # BASS_TRACE was requested but this container lacks the axon NTFF
        # profile hook (antenv.axon_hooks) — rerun with tracing suppressed.
        import os
        os.environ["BASS_NEVER_TRACE"] = "1"
        res = run_bass_kernel_spmd(nc, in_maps, core_ids=list(range(N_CORES)))
    global last_results
    last_results = res

    agg = np.empty((B, L, D_IN), np.float32)
    for c in range(N_CORES):
        img, qoff = c // 4, (c % 4) * QS
        a = res.results[c]["aggT"]
        rden = 1.0 / np.maximum(a[D_IN, :], 1e-30)
        agg[img, qoff:qoff + QS, :] = (a[:D_IN, :] * rden[None, :]).T

    zi = _fold(agg)
    out = b + _conv1x1(zi, np.asarray(W_w), np.asarray(W_b))
    return out.astype(np.float32)
